# revision 14
# baseline (speedup 1.0000x reference)
# Trainium2 Bass kernel for nn_Attention (4x2048x1024, H=16, DH=64) on 8
# NeuronCores. Sharding: core c = 2*bi + g handles batch bi and head group g
# (8 of 16 heads); host sums the two partial-MLP outputs per batch + bias.
#
# v4: software-pipelined bf16 kernel.
# Body half k: attention+MLP on buffer set CUR while QKV chains for the next
# iteration fill set NXT, interleaved between attention head-pair groups.
# - S-pairs row-paired via tile_position; exp on ACT is the critical engine
#   (~266us/iter); QKV/MLP chains fill PE slack during exp waits.
# - VA layout per key-tile: 4 pairs x 192 cols: [V_even |ones| V_odd]; the
#   shared ones block gives PV denominator rows for both heads of the pair.
# - All QKV/V/MLP psum chains share one [128,1024] f32 psum tag (2 banks,
#   bufs=1): A-half cols 0:512, B-half 512:1024, DVE copy/add afterwards.
# - psum: sps 4 banks + ops 2 + big 2 = 8.
import numpy as np
import concourse.bass as bass
import concourse.mybir as mybir
import concourse.tile as tile
from concourse import bacc, bass_utils

f32 = mybir.dt.float32
bf16 = mybir.dt.bfloat16
AF = mybir.ActivationFunctionType
NPBF16 = mybir.dt.np(bf16)

TOK = 2048
DIM = 1024
NH = 8
DH = 64
FEAT = NH * DH
KT = DIM // 128
TT = TOK // 128
NQC = TOK // 512
HP = NH // 2


def build(reps=1):
    assert isinstance(reps, str) or reps == 1 or reps % 2 == 0
    nc = bacc.Bacc("TRN2", target_bir_lowering=False, debug=False)
    xt = nc.dram_tensor("xt", [DIM, TOK], bf16, kind="ExternalInput").ap()
    wq = nc.dram_tensor("wq", [DIM, FEAT], bf16, kind="ExternalInput").ap()
    wk = nc.dram_tensor("wk", [DIM, FEAT], bf16, kind="ExternalInput").ap()
    wv = nc.dram_tensor("wv", [DIM, FEAT], bf16, kind="ExternalInput").ap()
    wm = nc.dram_tensor("wm", [FEAT, DIM], bf16, kind="ExternalInput").ap()
    outT = nc.dram_tensor("outT", [DIM, TOK], bf16, kind="ExternalOutput").ap()

    with tile.TileContext(nc) as tc:
        with tc.tile_pool(name="const", bufs=1) as constp, \
             tc.tile_pool(name="qkvA", bufs=1) as qkvA, \
             tc.tile_pool(name="qkvB", bufs=1) as qkvB, \
             tc.tile_pool(name="xtp", bufs=1) as xtp, \
             tc.tile_pool(name="wqkvp", bufs=1) as wqkvp, \
             tc.tile_pool(name="wmpool", bufs=1) as wmp, \
             tc.tile_pool(name="pt", bufs=4) as ptp, \
             tc.tile_pool(name="rcp", bufs=1) as rcp, \
             tc.tile_pool(name="arp", bufs=1) as arp, \
             tc.tile_pool(name="mcp", bufs=1) as mcp, \
             tc.tile_pool(name="mev", bufs=4) as mev, \
             tc.tile_pool(name="sps", bufs=1, space="PSUM") as sps, \
             tc.tile_pool(name="ops", bufs=1, space="PSUM") as ops, \
             tc.tile_pool(name="bigp", bufs=1, space="PSUM") as bigp:
            onesf = constp.tile([128, 256], f32, name="onesf")
            nc.gpsimd.memset(onesf[:], 1.0)
            onesb = constp.tile([128, 256], bf16, name="onesb")
            nc.vector.tensor_copy(onesb[:], onesf[:])
            ones4 = onesb[:].rearrange("p (h e) -> p h e", e=64)

            sets = []
            for nm, pool in (("A", qkvA), ("B", qkvB)):
                QT = [pool.tile([128, TOK], bf16, name=f"QT{nm}{i}") for i in range(4)]
                KTt = [pool.tile([128, TOK], bf16, name=f"KT{nm}{i}") for i in range(4)]
                VA = [pool.tile([128, HP * 192], bf16, name=f"VA{nm}{i}") for i in range(TT)]
                for tt in range(TT):
                    va_v = VA[tt][:].rearrange("p (r e) -> p r e", e=192)
                    nc.vector.tensor_copy(va_v[:, :, 64:128], ones4)
                sets.append({"QT": QT, "KT": KTt, "VA": VA})

            xT = [xtp.tile([128, TOK], bf16, name=f"xT{k}") for k in range(KT)]
            wqr = [wqkvp.tile([128, FEAT], bf16, name=f"wqr{k}") for k in range(KT)]
            wkr = [wqkvp.tile([128, FEAT], bf16, name=f"wkr{k}") for k in range(KT)]
            wvr = [wqkvp.tile([128, FEAT], bf16, name=f"wvr{k}") for k in range(KT)]
            wmrp = [wmp.tile([128, DIM], bf16, name=f"wmrp{p}") for p in range(HP)]

            def emit_w_dmas():
                for src, dst in ((wq, wqr), (wk, wkr), (wv, wvr)):
                    for k in range(KT):
                        nc.sync.dma_start(out=dst[k][:], in_=src[k * 128:(k + 1) * 128, :])
                for p in range(HP):
                    nc.sync.dma_start(out=wmrp[p][:], in_=wm[p * 128:(p + 1) * 128, :])

            def emit_dmas():
                # weights stay resident across loop iterations; only x reloads
                for k in range(KT):
                    nc.sync.dma_start(out=xT[k][:], in_=xt[k * 128:(k + 1) * 128, :])

            def qk_unit_chunks(dst_set, which, qc, fp):
                # 4 chunks of 4 matmuls each; big tile alloc deferred to chunk 0
                W = wqr if which == "q" else wkr
                dstl = dst_set["QT"] if which == "q" else dst_set["KT"]
                st = {}

                def mk(half, f, klo, copy_half):
                    def go():
                        if "big" not in st:
                            st["big"] = bigp.tile([128, 1024], f32, tag="big",
                                                  bufs=1, name="bigqk")
                        big = st["big"]
                        for k in range(klo, klo + 2):
                            nc.tensor.matmul(big[:, half * 512:(half + 1) * 512],
                                             W[k][:, f * 128:(f + 1) * 128],
                                             xT[k][:, qc * 512:(qc + 1) * 512],
                                             start=(k == 0), stop=(k == KT - 1))
                        if copy_half is not None:
                            h2, f2 = copy_half, (2 * fp if copy_half == 0 else 2 * fp + 1)
                            nc.vector.tensor_copy(
                                dstl[f2][:, qc * 512:(qc + 1) * 512],
                                big[:, h2 * 512:(h2 + 1) * 512])
                    return go
                return [mk(0, 2 * fp, 0, None), mk(0, 2 * fp, 2, None),
                        mk(0, 2 * fp, 4, None), mk(0, 2 * fp, 6, 0),
                        mk(1, 2 * fp + 1, 0, None), mk(1, 2 * fp + 1, 2, None),
                        mk(1, 2 * fp + 1, 4, None), mk(1, 2 * fp + 1, 6, 1)]

            def v_unit_chunks(dst_set, jp):
                st = {}

                def mk(half, klo, copy_half):
                    def go():
                        if "big" not in st:
                            st["big"] = bigp.tile([128, 1024], f32, tag="big",
                                                  bufs=1, name="bigv")
                        big = st["big"]
                        tt = 2 * jp + half
                        for k in range(klo, klo + 2):
                            nc.tensor.matmul(big[:, half * 512:(half + 1) * 512],
                                             xT[k][:, tt * 128:(tt + 1) * 128],
                                             wvr[k][:],
                                             start=(k == 0), stop=(k == KT - 1))
                        if copy_half is not None:
                            h2 = copy_half
                            t2 = 2 * jp + h2
                            va_v = dst_set["VA"][t2][:].rearrange(
                                "p (r e) -> p r e", e=192)
                            pv_v = big[:, h2 * 512:(h2 + 1) * 512].rearrange(
                                "p (h e) -> p h e", e=64)
                            nc.vector.tensor_copy(va_v[:, :, 0:64],
                                                  pv_v[:, 0::2, :])
                            nc.vector.tensor_copy(va_v[:, :, 128:192],
                                                  pv_v[:, 1::2, :])
                    return go
                return [mk(0, 0, None), mk(0, 2, None), mk(0, 4, None), mk(0, 6, 0),
                        mk(1, 0, None), mk(1, 2, None), mk(1, 4, None), mk(1, 6, 1)]

            def mlp_chunks(ar_pairs, qc, m):
                st = {}

                def half(plo, last):
                    def go():
                        if "big" not in st:
                            st["big"] = bigp.tile([128, 1024], f32, tag="big",
                                                  bufs=1, name="bigm")
                        pm = st["big"]
                        for p in (plo, plo + 1):
                            nc.tensor.matmul(pm[:, 0:512],
                                             wmrp[p][0:64, m * 128:(m + 1) * 128],
                                             ar_pairs[p][0:64, :],
                                             start=(p == 0), stop=(p == HP - 1),
                                             tile_position=(0, 0))
                            nc.tensor.matmul(pm[:, 512:1024],
                                             wmrp[p][64:128, m * 128:(m + 1) * 128],
                                             ar_pairs[p][64:128, :],
                                             start=(p == 0), stop=(p == HP - 1),
                                             tile_position=(64, 0))
                        if last:
                            mc = mcp.tile([128, 512], f32, tag="mc")
                            nc.vector.tensor_copy(mc[:], pm[:, 512:1024])
                            ev = mev.tile([128, 512], bf16, tag="ev")
                            nc.vector.tensor_add(ev[:], pm[:, 0:512], mc[:])
                            nc.sync.dma_start(
                                out=outT[m * 128:(m + 1) * 128,
                                         qc * 512:(qc + 1) * 512],
                                in_=ev[:])
                    return go
                return [half(0, False), half(2, True)]

            def emit_mlp(ar_pairs, qc, m):
                for c in mlp_chunks(ar_pairs, qc, m):
                    c()

            def emit_s_pair(cur, qc, hp, mt):
                QT, KTt = cur["QT"], cur["KT"]
                ps_s = sps.tile([128, 1024], f32, tag="s", bufs=2, name="spsn")
                nc.tensor.matmul(
                    ps_s[:, 0:512],
                    KTt[hp][0:64, mt * 128:(mt + 1) * 128],
                    QT[hp][0:64, qc * 512:(qc + 1) * 512],
                    start=True, stop=True, tile_position=(0, 0))
                nc.tensor.matmul(
                    ps_s[:, 512:1024],
                    KTt[hp][64:128, mt * 128:(mt + 1) * 128],
                    QT[hp][64:128, qc * 512:(qc + 1) * 512],
                    start=True, stop=True, tile_position=(64, 0))
                return ps_s

            def attn_half(cur, nxt, carry):
                # Build per-qc chunk streams: MLP m-tiles for the carried qc +
                # QKV/V unit chunks producing nxt. One chunk pops per g-block.
                unit_lists = []
                if nxt is not None:
                    for jp in range(8):
                        unit_lists.append(v_unit_chunks(nxt, jp))
                    for fp in range(2):
                        for qc in range(NQC):
                            unit_lists.append(qk_unit_chunks(nxt, "k", qc, fp))
                            unit_lists.append(qk_unit_chunks(nxt, "q", qc, fp))

                for qc in range(NQC):
                    # region stream: alternate MLP m-tiles (2 chunks + gap) and
                    # units (4 chunks + gap), preserving intra-unit order
                    region = []
                    mlp_m = list(range(8)) if carry["ar"] is not None else []
                    lo = qc * len(unit_lists) // NQC
                    hi = (qc + 1) * len(unit_lists) // NQC
                    myunits = unit_lists[lo:hi]
                    while mlp_m or myunits:
                        if mlp_m:
                            m = mlp_m.pop(0)
                            region += mlp_chunks(carry["ar"], carry["qc"], m)
                        if myunits:
                            region += myunits.pop(0)
                    ri = 0

                    for hp in range(HP):
                        poA = ops.tile([128, 512], f32, tag="oA", name="poA")
                        poB = ops.tile([128, 512], f32, tag="oB", name="poB")
                        s_cur = emit_s_pair(cur, qc, hp, 0)
                        for mt in range(TT):
                            pt2 = ptp.tile([128, 1024], bf16, tag="pt", name="pt2")
                            nc.scalar.activation(pt2[:], s_cur[:], AF.Exp, scale=0.125)
                            if mt < TT - 1:
                                s_cur = emit_s_pair(cur, qc, hp, mt + 1)
                            if ri < len(region):
                                c = region[ri]; ri += 1
                                if c is not None:
                                    c()
                            VA = cur["VA"]
                            nc.tensor.matmul(poA[:],
                                             VA[mt][:, hp * 192:hp * 192 + 128],
                                             pt2[:, 0:512],
                                             start=(mt == 0), stop=(mt == TT - 1))
                            nc.tensor.matmul(poB[:],
                                             VA[mt][:, hp * 192 + 64:hp * 192 + 192],
                                             pt2[:, 512:1024],
                                             start=(mt == 0), stop=(mt == TT - 1))
                        # normalize -> ar
                        dn = rcp.tile([128, 512], f32, tag="dn", name="dn")
                        nc.vector.tensor_copy(dn[0:64, :], poA[64:128, :])
                        nc.vector.tensor_copy(dn[64:128, :], poB[0:64, :])
                        rc = rcp.tile([128, 512], f32, tag="rc", name="rc")
                        nc.vector.reciprocal_approx_fast(out=rc[:], in_=dn[:])
                        arP = arp.tile([128, 512], bf16, tag=f"ar{hp}", bufs=3,
                                       name=f"arP{hp}")
                        nc.vector.tensor_mul(arP[0:64, :], poA[0:64, :], rc[0:64, :])
                        nc.vector.tensor_mul(arP[64:128, :], poB[64:128, :],
                                             rc[64:128, :])
                        carry["hp_ar"][hp] = arP
                    # drain any leftover chunks of this region
                    while ri < len(region):
                        c = region[ri]; ri += 1
                        if c is not None:
                            c()
                    carry["ar"] = list(carry["hp_ar"])
                    carry["qc"] = qc
                    carry["m_left"] = []
                return carry

            def p1_serial(dst_set):
                emit_w_dmas()
                emit_dmas()
                for jp in range(8):
                    for c in v_unit_chunks(dst_set, jp):
                        c()
                for fp in range(2):
                    for qc in range(NQC):
                        for c in qk_unit_chunks(dst_set, "k", qc, fp):
                            c()
                        for c in qk_unit_chunks(dst_set, "q", qc, fp):
                            c()

            carry = {"ar": None, "qc": 0, "m_left": [], "hp_ar": [None] * HP}
            if isinstance(reps, str) and reps.startswith("flat"):
                nh = int(reps[4:])
                p1_serial(sets[0])
                for i in range(nh):
                    emit_dmas()
                    attn_half(sets[i % 2], sets[(i + 1) % 2], carry)
            elif reps == 1:
                p1_serial(sets[0])
                attn_half(sets[0], None, carry)
            else:
                p1_serial(sets[0])
                with tc.For_i(0, reps // 2, 1):
                    emit_dmas()
                    attn_half(sets[0], sets[1], carry)
                    emit_dmas()
                    attn_half(sets[1], sets[0], carry)
            # flush pending MLP of the final qc
            if carry["ar"] is not None:
                for m in range(8):
                    emit_mlp(carry["ar"], carry["qc"], m)
    nc.compile()
    return nc


_nc_cache = {}


def get_nc(reps=1):
    if reps not in _nc_cache:
        _nc_cache[reps] = build(reps)
    return _nc_cache[reps]


def make_in_maps(input, W_qkv, W_mlp):
    in_maps = []
    for c in range(8):
        bi, g = c // 2, c % 2
        cols = slice(g * FEAT, (g + 1) * FEAT)
        in_maps.append({
            "xt": np.ascontiguousarray(input[bi].T).astype(NPBF16),
            "wq": np.ascontiguousarray(W_qkv[:, 0 * DIM:1 * DIM][:, cols]).astype(NPBF16),
            "wk": np.ascontiguousarray(W_qkv[:, 1 * DIM:2 * DIM][:, cols]).astype(NPBF16),
            "wv": np.ascontiguousarray(W_qkv[:, 2 * DIM:3 * DIM][:, cols]).astype(NPBF16),
            "wm": np.ascontiguousarray(W_mlp[g * FEAT:(g + 1) * FEAT, :]).astype(NPBF16),
        })
    return in_maps


def kernel(input, W_qkv, W_mlp, b_mlp, reps=1):
    nc = get_nc(reps)
    in_maps = make_in_maps(np.asarray(input), np.asarray(W_qkv), np.asarray(W_mlp))
    res = bass_utils.run_bass_kernel_spmd(nc, in_maps, core_ids=list(range(8)))
    out = np.empty((4, TOK, DIM), np.float32)
    b = np.asarray(b_mlp)
    for bi in range(4):
        out[bi] = (res.results[2 * bi]["outT"].astype(np.float32)
                   + res.results[2 * bi + 1]["outT"].astype(np.float32)).T + b
    return out
